# revision 33
# baseline (speedup 1.0000x reference)
"""Trainium2 Bass kernel for per-node LocalConv1D (kernel_size=1).

out[b, o, n] = sum_h W[n, o, h] * x[b, h, n] + b[n, o]

Full shapes: x [16, 32, 50000] f32, W [50000, 32, 32] f32, b [50000, 32] f32,
out [16, 32, 50000] f32.

Sharding: node dim n split evenly across 8 NeuronCores (6250 nodes/core,
zero-padded to 6272 inside each shard). Fully independent per-node 32x32
matmuls -> no collectives.

Per-core device strategy (fp16 data path, ~26 MB HBM traffic/core):

  Nodes are processed in GROUPS of 4: group s covers nodes {4s+k}. The four
  nodes' weights are stacked along the PE contraction dim, giving a DENSE
  32-column stationary operand (8 weight columns per node instead of 32):

      lhsT[32k+h, o] = W[4s+k, o, h]            (128 x 32, no zeros)

  The moving operand separates the nodes again: 64 columns (k, b) where
  partition rows 32k'+h carry x[b, h, 4s+k] iff k' == k and ZERO otherwise.
  The zeros live in two persistent SBUF x-buffers, written once at startup
  by memzero ops spread across DVE/ACT; DMA only ever rewrites the
  block-diagonal rectangles (4 dense sub-rectangle DMAs per chunk, ramped
  chunk sizes so the first fill is tiny and compute starts early).

      out[o, (k, b)] = sum_{k',h} lhsT[32k'+h, o] * rhs[32k'+h, (k,b)]
                     = sum_h W[4s+k, o, h] x[b, h, 4s+k]        (exact)

  Each group's 32x64 result goes to PSUM column strip c = s % 4 via
  tile_position=(0, 32c), so 4 consecutive groups (a "super" of 16 nodes)
  fill a full [128, 64] PSUM region, and 8 supers fill one 2 KiB PSUM bank
  (128 nodes per bank, 49 banks per core). Eviction is one DVE tensor_add
  per bank which also adds the bias (resident fp16 slab, broadcast over b
  with a stride-0 AP dim) and converts to fp16.

  PE cost per group: 32-column LDWEIGHTS + 64-column MATMUL (~2 x 27 ns),
  1568 groups/core. DMA: W is a fully resident dense [128, 50176] fp16
  slab (~1MB chunks on the sync HWDGE ring, bias first since it gates the
  first eviction); x streams through the ping-pong buffers on the gpsimd
  SWDGE ring so its WAR waits never block W; out stores are 8-bank windows
  on the scalar HWDGE ring, tapered at the kernel tail. Measured ~115 us
  on 8 axon-tunneled TRN2 cores (vs 26.1 MB/core compulsory HBM traffic
  ~73 us + ~15 us fixed preamble/drain).
"""

from contextlib import ExitStack

import numpy as np

import concourse.bass as bass
import concourse.mybir as mybir
import concourse.tile as tile

F16 = mybir.dt.float16
F32 = mybir.dt.float32

B = 16  # batch
H = 32  # in channels
O = 32  # out channels
NCORES = 8
NFULL = 50000
NPC = NFULL // NCORES  # 6250 nodes per core
NPAD = 6272  # padded per-core node count
NG = NPAD // 4  # 1568 groups of 4 nodes
NSUP = NG // 4  # 392 supers of 16 nodes
NB = NPAD // 128  # 49 PSUM-bank rounds (8 supers each)
XCOLS = 224 * B  # 3584 cols per x-buffer k-region (7 bank rounds)
WCOLS = NG * O  # 50176 W slab cols
OUTCOLS = NPAD * B * O // 128  # 25088 out cols
# W load col edges: small first chunk so compute starts fast, 1 MB after
W_EDGES = [0, 2048] + list(range(6144, WCOLS, 4096)) + [WCOLS]
# out store windows (banks): big in the middle, tapered at the end so the
# final store is tiny and the drain starts early
OW_EDGES = [0, 8, 16, 24, 32, 40, 45, 48, 49]

# x chunking: bank-round edges, target buffer (0=A, 1=B), strip col offset.
# Ramped chunk sizes [1,2,4,5,7,...] so the zero-fill gating compute start
# is tiny; early chunks live in disjoint strips of the two buffers.
XR_EDGES = [0, 1, 3, 7, 12, 19, 26, 33, 40, 47, 49]
XBUF_OF = [0, 1, 0, 1, 0, 1, 0, 1, 0, 1]
XSTRIP = [0, 0, 512, 1024, 0, 0, 0, 0, 0, 0]
XNCH = len(XBUF_OF)


def _xdiag_dma(nc, xbufs, x_d, ch):
    """Issue the 4 diagonal-rectangle DMAs for x chunk `ch`.

    On the scalar HWDGE ring (shared with out stores): SWDGE costs ~1 us of
    serial Q7 descriptor-generation per dma_start, which made every chunk
    arrive late and stalled the PE at each chunk boundary."""
    g0, g1 = 32 * XR_EDGES[ch], 32 * XR_EDGES[ch + 1]
    dst = xbufs[XBUF_OF[ch]]
    so = XSTRIP[ch]
    w = (g1 - g0) * B
    for k in range(4):
        nc.scalar.dma_start(
            out=dst[32 * k : 32 * k + 32, k * XCOLS + so : k * XCOLS + so + w],
            in_=x_d[k, :, g0 * B : g1 * B],
        )


def build_bass():
    nc = bass.Bass()
    w_d = nc.declare_dram_parameter("W", [128, WCOLS], F16, isOutput=False)
    x_d = nc.declare_dram_parameter("x", [4, 32, NG * B], F16, isOutput=False)
    b_d = nc.declare_dram_parameter("b", [128, NG], F16, isOutput=False)
    out_d = nc.declare_dram_parameter("out", [128, OUTCOLS], F16, isOutput=True)

    with ExitStack() as ctx:
        tc = ctx.enter_context(tile.TileContext(nc))
        wp = ctx.enter_context(tc.tile_pool(name="wp", bufs=1))
        xp = ctx.enter_context(tc.tile_pool(name="xp", bufs=1))
        bp = ctx.enter_context(tc.tile_pool(name="bp", bufs=1))
        op = ctx.enter_context(tc.tile_pool(name="op", bufs=2))
        pp = ctx.enter_context(tc.tile_pool(name="pp", bufs=8, space="PSUM"))

        # resident dense weight slab on the sync HWDGE ring. The bias gates
        # the first eviction (~13 us in) so it slots after the first two W
        # chunks — early enough, without delaying W chunk 0 (which gates
        # compute start) — but never behind the full 12.8 MB of W.
        bt = bp.tile([128, NG], F16)
        wt = wp.tile([128, WCOLS], F16)
        for wc, (c0, c1) in enumerate(zip(W_EDGES[:-1], W_EDGES[1:])):
            nc.sync.dma_start(out=wt[:, c0:c1], in_=w_d[:, c0:c1])
            if wc == 1:
                nc.sync.dma_start(out=bt[:], in_=b_d[:])

        # ping-pong x buffers; zeros off the block diagonal are persistent
        # (zero-filled once; DMA only ever rewrites the diagonal rectangles).
        # The fills are memzero (uint32-bitcast mul, ~2x faster than memset)
        # spread over DVE / ACT / GpSimd, ordered so the only one gating
        # compute start is chunk 0's tiny 512-col strip.
        xbuf_a = xp.tile([128, 4 * XCOLS], F16, tag="xa")
        xbuf_b = xp.tile([128, 4 * XCOLS], F16, tag="xb")
        xbufs = [xbuf_a, xbuf_b]

        def _zero(eng, buf, c_lo, c_hi, ks):
            for k in ks:
                eng.memzero(xbufs[buf][:, k * XCOLS + c_lo : k * XCOLS + c_hi])

        # DVE gates the first two chunks; GpSimd (whose Q7 no longer does
        # any DMA descriptor generation) fills the rest. ACT stays purely a
        # DMA-trigger stream so x chunks are never queued behind compute.
        _zero(nc.vector, 0, 0, 512, range(4))  # gates chunk 0
        _zero(nc.vector, 1, 0, 1024, range(4))  # gates chunk 1
        _zero(nc.gpsimd, 0, 512, 2560, range(4))  # gates chunk 2
        _zero(nc.gpsimd, 1, 1024, 3584, range(4))  # gates chunk 3
        _zero(nc.gpsimd, 0, 2560, 3584, range(4))  # gates chunk 4 (round 12)
        for c in range(4):
            _xdiag_dma(nc, xbufs, x_d, c)

        ot = None
        ow = 0
        ch = 0
        for j in range(NB):  # bank rounds: 8 supers = 32 groups = 128 nodes
            if j == XR_EDGES[ch + 1]:
                ch += 1
            # prefetch x chunk ch+1 at the first round of chunk ch's window
            # (chunks 0-3 were issued before the loop). ch+1 targets the
            # OPPOSITE buffer, whose previous readers (chunk ch-1) are fully
            # traced by now — issuing a same-buffer chunk here would make
            # this chunk's own not-yet-traced MMs read the new data.
            if ch >= 3 and ch + 1 < XNCH and j == XR_EDGES[ch]:
                _xdiag_dma(nc, xbufs, x_d, ch + 1)

            xv = xbufs[XBUF_OF[ch]][:].rearrange("p (k u) -> p k u", k=4)
            ps = pp.tile([128, 512], F32)
            ps_v = ps[:].rearrange("p (q k b) -> p q k b", q=8, k=4, b=B)

            for p in range(8):  # supers within the bank
                g4 = 8 * j + p
                for c in range(4):
                    s = 4 * g4 + c  # global group
                    xo = XSTRIP[ch] + (s - 32 * XR_EDGES[ch]) * B
                    nc.tensor.matmul(
                        ps_v[32 * c : 32 * c + 32, p, :, :],
                        wt[:, O * s : O * s + O],
                        xv[:, :, xo : xo + B],
                        start=True,
                        stop=True,
                        tile_position=(0, 32 * c),
                    )

            # eviction + bias add -> fp16 out tile
            if j == OW_EDGES[ow]:
                ot = op.tile([128, 8 * 512], F16, tag="ot")
            jo = (j - OW_EDGES[ow]) * 512
            out_v = ot[:, jo : jo + 512].rearrange("p (g b) -> p g b", g=32)
            bias_v = (
                bt[:, 32 * j : 32 * j + 32]
                .unsqueeze(2)
                .broadcast_to([128, 32, B])
            )
            ps_flat = ps[:].rearrange("p (g b) -> p g b", g=32)
            nc.vector.tensor_add(out_v, ps_flat, bias_v)

            if j == OW_EDGES[ow + 1] - 1:
                w0 = OW_EDGES[ow] * 512
                wn = (OW_EDGES[ow + 1] - OW_EDGES[ow]) * 512
                nc.scalar.dma_start(
                    out=out_d[:, w0 : w0 + wn], in_=ot[:, :wn]
                )
                ow += 1

    return nc


def _legalize_waits(nc):
    """Walrus's per-instruction sync structs carry at most one wait
    (DMA_DIRECT2D, S3_LW, ...); Tile sometimes leaves several on one
    instruction. Move the surplus onto EventSemaphore instructions inserted
    just before it on the same engine — the issuing sequencer executes its
    stream in order, so the waits still gate the instruction."""
    nsplit = 0
    for f in nc.m.functions:
        for bb in f.blocks:
            new = []
            changed = False
            for inst in bb.instructions:
                si = getattr(inst, "sync_info", None)
                if (
                    si is not None
                    and si.on_wait
                    and len(si.on_wait) > 1
                    and type(inst).__name__ != "InstEventSemaphore"
                ):
                    waits = list(si.on_wait)
                    for w in waits[:-1]:
                        nsplit += 1
                        new.append(
                            mybir.InstEventSemaphore(
                                name=f"wait-split-{nsplit}",
                                engine=inst.engine,
                                ins=[],
                                outs=[],
                                sync_info=mybir.SyncInfo(
                                    on_wait=[w], on_update=[]
                                ),
                            )
                        )
                    inst.sync_info = mybir.SyncInfo(
                        on_wait=[waits[-1]], on_update=list(si.on_update)
                    )
                    changed = True
                new.append(inst)
            if changed:
                bb.instructions = new
    return nc


_NC_CACHE = {}


def _get_nc():
    if "nc" not in _NC_CACHE:
        _NC_CACHE["nc"] = _legalize_waits(build_bass())
    return _NC_CACHE["nc"]


def prep_core_inputs(x_s, W_s, b_s):
    """Per-core shard [*, NPC nodes] -> device-layout arrays (padded)."""
    xs = np.zeros((B, H, NPAD), np.float16)
    xs[:, :, :NPC] = x_s
    Ws = np.zeros((NPAD, O, H), np.float32)
    Ws[:NPC] = W_s
    bs = np.zeros((NPAD, O), np.float32)
    bs[:NPC] = b_s

    # W slab [128, WCOLS]: [32k+h, 32s+o] = W[4s+k, o, h]
    wslab = np.ascontiguousarray(
        Ws.reshape(NG, 4, O, H).transpose(1, 3, 0, 2).reshape(128, WCOLS)
    ).astype(np.float16)

    # x slab [4, 32, NG*B]: [k, h, 16*s + b] = x[b, h, 4s+k]
    xd = np.ascontiguousarray(
        xs.reshape(B, H, NG, 4).transpose(3, 1, 2, 0).reshape(4, 32, NG * B)
    )

    # bias slab [128, NG]: [32c+o, 4*g4+k] = b[16g4+4c+k, o]
    bslab = np.ascontiguousarray(
        bs.reshape(NSUP, 4, 4, O).transpose(1, 3, 0, 2).reshape(128, NG)
    ).astype(np.float16)

    return {"x": xd, "W": wslab, "b": bslab}


def unprep_core_output(op):
    """Device out slab [128, OUTCOLS] fp16 -> [B, O, NPC] f32."""
    # [32c+o, 64*g4 + 16k + b] = out[b, o, 16g4+4c+k]
    arr = np.asarray(op).reshape(4, O, NSUP, 4, B).transpose(4, 1, 2, 0, 3)
    return arr.reshape(B, O, NPAD)[:, :, :NPC].astype(np.float32)


def make_in_maps(x, W, b):
    x = np.ascontiguousarray(x, dtype=np.float32)
    W = np.ascontiguousarray(W, dtype=np.float32)
    b = np.ascontiguousarray(b, dtype=np.float32)
    in_maps = []
    for core in range(NCORES):
        sl = slice(core * NPC, (core + 1) * NPC)
        in_maps.append(prep_core_inputs(x[:, :, sl], W[sl], b[sl]))
    return in_maps


def run_spmd(in_maps, **kwargs):
    from concourse.bass_utils import run_bass_kernel_spmd

    nc = _get_nc()
    return run_bass_kernel_spmd(
        nc, in_maps, core_ids=list(range(NCORES)), **kwargs
    )


def kernel(x, W, b):
    res = run_spmd(make_in_maps(x, W, b))
    out = np.concatenate(
        [unprep_core_output(res.results[c]["out"]) for c in range(NCORES)],
        axis=2,
    )
    return out


# revision 40
# speedup vs baseline: 1.0078x; 1.0078x over previous
"""Trainium2 Bass kernel for per-node LocalConv1D (kernel_size=1).

out[b, o, n] = sum_h W[n, o, h] * x[b, h, n] + b[n, o]

Full shapes: x [16, 32, 50000] f32, W [50000, 32, 32] f32, b [50000, 32] f32,
out [16, 32, 50000] f32.

Sharding: node dim n split evenly across 8 NeuronCores (6250 nodes/core,
zero-padded to 6272 inside each shard). Fully independent per-node 32x32
matmuls -> no collectives.

Per-core device strategy (fp16 data path, ~26 MB HBM traffic/core):

  Nodes are processed in GROUPS of 4: group s covers nodes {4s+k}. The four
  nodes' weights are stacked along the PE contraction dim, giving a DENSE
  32-column stationary operand (8 weight columns per node instead of 32):

      lhsT[32k+h, o] = W[4s+k, o, h]            (128 x 32, no zeros)

  The moving operand separates the nodes again: 64 columns (k, b) where
  partition rows 32k'+h carry x[b, h, 4s+k] iff k' == k and ZERO otherwise.
  The zeros live in two persistent SBUF x-buffers, written once at startup
  by memzero ops spread across DVE/ACT; DMA only ever rewrites the
  block-diagonal rectangles (4 dense sub-rectangle DMAs per chunk, ramped
  chunk sizes so the first fill is tiny and compute starts early).

      out[o, (k, b)] = sum_{k',h} lhsT[32k'+h, o] * rhs[32k'+h, (k,b)]
                     = sum_h W[4s+k, o, h] x[b, h, 4s+k]        (exact)

  Each group's 32x64 result goes to PSUM column strip c = s % 4 via
  tile_position=(0, 32c), so 4 consecutive groups (a "super" of 16 nodes)
  fill a full [128, 64] PSUM region, and 8 supers fill one 2 KiB PSUM bank
  (128 nodes per bank, 49 banks per core). Eviction is one DVE tensor_add
  per bank which also adds the bias (resident fp16 slab, broadcast over b
  with a stride-0 AP dim) and converts to fp16.

  PE cost per group: 32-column LDWEIGHTS + 64-column MATMUL (~2 x 27 ns),
  1568 groups/core. DMA: W is a fully resident dense [128, 50176] fp16
  slab (~1MB chunks on the sync HWDGE ring, bias first since it gates the
  first eviction); x streams through the ping-pong buffers on the gpsimd
  SWDGE ring so its WAR waits never block W; out stores are 8-bank windows
  on the scalar HWDGE ring, tapered at the kernel tail. Measured ~115 us
  on 8 axon-tunneled TRN2 cores (vs 26.1 MB/core compulsory HBM traffic
  ~73 us + ~15 us fixed preamble/drain).
"""

from contextlib import ExitStack

import numpy as np

import concourse.bass as bass
import concourse.mybir as mybir
import concourse.tile as tile

F16 = mybir.dt.float16
F32 = mybir.dt.float32

B = 16  # batch
H = 32  # in channels
O = 32  # out channels
NCORES = 8
NFULL = 50000
NPC = NFULL // NCORES  # 6250 nodes per core
NPAD = 6272  # padded per-core node count
NG = NPAD // 4  # 1568 groups of 4 nodes
NSUP = NG // 4  # 392 supers of 16 nodes
NB = NPAD // 128  # 49 PSUM-bank rounds (8 supers each)
XCOLS = 192 * B  # 3072 cols per x-buffer k-region (6 bank rounds)
WCOLS = NG * O  # 50176 W slab cols
OUTCOLS = NPAD * B * O // 128  # 25088 out cols
# W load col edges: small first chunk so compute starts fast, 1 MB after
W_EDGES = [0, 2048] + list(range(6144, WCOLS, 4096)) + [WCOLS]
# out store windows (banks): big in the middle, tapered at the end so the
# final store is tiny and the drain starts early
OW_EDGES = [0, 8, 16, 24, 32, 40, 45, 48, 49]

# x chunking: bank-round edges and target buffer (0=A, 1=B, 2=C).
# THREE rotating buffers give prefetch distance 2 under Tile's program-
# order WAR rule: chunk c is issued at the first round of window c-2, by
# which point its same-buffer predecessor c-3's readers are all traced.
# Ramped early chunk sizes [1,2,4] so the zero-fill gating compute start
# is tiny.
XR_EDGES = [0, 1, 3, 7, 13, 19, 25, 31, 37, 43, 49]
XBUF_OF = [0, 1, 2, 0, 1, 2, 0, 1, 2, 0]
XSTRIP = [0] * 10
XNCH = len(XBUF_OF)


def _xdiag_dma(nc, xbufs, x_d, ch, eng=None):
    """Issue the 4 diagonal-rectangle DMAs for x chunk `ch`.

    In-loop chunks go on the gpsimd SWDGE ring so their WAR waits never
    head-of-line block the W or out HWDGE queues; the upfront chunks 0-2
    (no WAR waits, but latency-critical for compute start) go on the
    scalar HWDGE ring to skip the ~1 us/DMA serial Q7 descriptor-gen."""
    g0, g1 = 32 * XR_EDGES[ch], 32 * XR_EDGES[ch + 1]
    dst = xbufs[XBUF_OF[ch]]
    so = XSTRIP[ch]
    w = (g1 - g0) * B
    for k in range(4):
        (eng or nc.gpsimd).dma_start(
            out=dst[32 * k : 32 * k + 32, k * XCOLS + so : k * XCOLS + so + w],
            in_=x_d[k, :, g0 * B : g1 * B],
        )


def build_bass():
    nc = bass.Bass()
    w_d = nc.declare_dram_parameter("W", [128, WCOLS], F16, isOutput=False)
    x_d = nc.declare_dram_parameter("x", [4, 32, NG * B], F16, isOutput=False)
    b_d = nc.declare_dram_parameter("b", [128, NG], F16, isOutput=False)
    out_d = nc.declare_dram_parameter("out", [128, OUTCOLS], F16, isOutput=True)

    with ExitStack() as ctx:
        tc = ctx.enter_context(tile.TileContext(nc))
        wp = ctx.enter_context(tc.tile_pool(name="wp", bufs=1))
        xp = ctx.enter_context(tc.tile_pool(name="xp", bufs=1))
        bp = ctx.enter_context(tc.tile_pool(name="bp", bufs=1))
        op = ctx.enter_context(tc.tile_pool(name="op", bufs=2))
        pp = ctx.enter_context(tc.tile_pool(name="pp", bufs=8, space="PSUM"))

        # resident dense weight slab on the sync HWDGE ring. The bias gates
        # the first eviction (~13 us in) so it slots after the first two W
        # chunks — early enough, without delaying W chunk 0 (which gates
        # compute start) — but never behind the full 12.8 MB of W.
        bt = bp.tile([128, NG], F16)
        wt = wp.tile([128, WCOLS], F16)
        for wc, (c0, c1) in enumerate(zip(W_EDGES[:-1], W_EDGES[1:])):
            nc.sync.dma_start(out=wt[:, c0:c1], in_=w_d[:, c0:c1])
            if wc == 1:
                nc.sync.dma_start(out=bt[:], in_=b_d[:])

        # ping-pong x buffers; zeros off the block diagonal are persistent
        # (zero-filled once; DMA only ever rewrites the diagonal rectangles).
        # The fills are memzero (uint32-bitcast mul, ~2x faster than memset)
        # spread over DVE / ACT / GpSimd, ordered so the only one gating
        # compute start is chunk 0's tiny 512-col strip.
        xbuf_a = xp.tile([128, 4 * XCOLS], F16, tag="xa")
        xbuf_b = xp.tile([128, 4 * XCOLS], F16, tag="xb")
        xbuf_c = xp.tile([128, 4 * XCOLS], F16, tag="xc")
        xbufs = [xbuf_a, xbuf_b, xbuf_c]

        def _zero(eng, buf, c_lo, c_hi, ks):
            for k in ks:
                eng.memzero(xbufs[buf][:, k * XCOLS + c_lo : k * XCOLS + c_hi])

        _zero(nc.vector, 0, 0, 512, range(4))  # gates chunk 0
        _zero(nc.vector, 1, 0, 1024, range(4))  # gates chunk 1
        _zero(nc.vector, 2, 0, 2048, (0, 1))  # gates chunk 2 (with ACT)
        _zero(nc.scalar, 2, 0, 2048, (2, 3))
        _zero(nc.scalar, 0, 512, 3072, range(4))  # gates chunk 3 reads
        _zero(nc.scalar, 1, 1024, 3072, range(4))  # gates chunk 4 reads
        _zero(nc.scalar, 2, 2048, 3072, range(4))  # gates chunk 5 reads
        # GpSimd ring: early chunk diagonals (keep the Q7 free for SWDGE
        # descriptor generation)
        for c in range(3):
            _xdiag_dma(nc, xbufs, x_d, c)

        ot = None
        ow = 0
        ch = 0
        for j in range(NB):  # bank rounds: 8 supers = 32 groups = 128 nodes
            if j == XR_EDGES[ch + 1]:
                ch += 1
            # prefetch x chunk ch+2 at the first round of chunk ch's window
            # (chunks 0-2 were issued before the loop). ch+2's buffer was
            # last used by chunk ch-1, whose readers are all traced by now
            # — never prefetch a chunk whose buffer's readers are still
            # untraced, or those MMs would be ordered after the new data.
            if ch >= 1 and ch + 2 < XNCH and j == XR_EDGES[ch]:
                _xdiag_dma(nc, xbufs, x_d, ch + 2)

            xv = xbufs[XBUF_OF[ch]][:].rearrange("p (k u) -> p k u", k=4)
            ps = pp.tile([128, 512], F32)
            ps_v = ps[:].rearrange("p (q k b) -> p q k b", q=8, k=4, b=B)

            for p in range(8):  # supers within the bank
                g4 = 8 * j + p
                for c in range(4):
                    s = 4 * g4 + c  # global group
                    xo = XSTRIP[ch] + (s - 32 * XR_EDGES[ch]) * B
                    nc.tensor.matmul(
                        ps_v[32 * c : 32 * c + 32, p, :, :],
                        wt[:, O * s : O * s + O],
                        xv[:, :, xo : xo + B],
                        start=True,
                        stop=True,
                        tile_position=(0, 32 * c),
                    )

            # eviction + bias add -> fp16 out tile
            if j == OW_EDGES[ow]:
                ot = op.tile([128, 8 * 512], F16, tag="ot")
            jo = (j - OW_EDGES[ow]) * 512
            out_v = ot[:, jo : jo + 512].rearrange("p (g b) -> p g b", g=32)
            bias_v = (
                bt[:, 32 * j : 32 * j + 32]
                .unsqueeze(2)
                .broadcast_to([128, 32, B])
            )
            ps_flat = ps[:].rearrange("p (g b) -> p g b", g=32)
            nc.vector.tensor_add(out_v, ps_flat, bias_v)

            if j == OW_EDGES[ow + 1] - 1:
                w0 = OW_EDGES[ow] * 512
                wn = (OW_EDGES[ow + 1] - OW_EDGES[ow]) * 512
                nc.scalar.dma_start(
                    out=out_d[:, w0 : w0 + wn], in_=ot[:, :wn]
                )
                ow += 1

    return nc


def _legalize_waits(nc):
    """Walrus's per-instruction sync structs carry at most one wait
    (DMA_DIRECT2D, S3_LW, ...); Tile sometimes leaves several on one
    instruction. Move the surplus onto EventSemaphore instructions inserted
    just before it on the same engine — the issuing sequencer executes its
    stream in order, so the waits still gate the instruction."""
    nsplit = 0
    for f in nc.m.functions:
        for bb in f.blocks:
            new = []
            changed = False
            for inst in bb.instructions:
                si = getattr(inst, "sync_info", None)
                if (
                    si is not None
                    and si.on_wait
                    and len(si.on_wait) > 1
                    and type(inst).__name__ != "InstEventSemaphore"
                ):
                    waits = list(si.on_wait)
                    for w in waits[:-1]:
                        nsplit += 1
                        new.append(
                            mybir.InstEventSemaphore(
                                name=f"wait-split-{nsplit}",
                                engine=inst.engine,
                                ins=[],
                                outs=[],
                                sync_info=mybir.SyncInfo(
                                    on_wait=[w], on_update=[]
                                ),
                            )
                        )
                    inst.sync_info = mybir.SyncInfo(
                        on_wait=[waits[-1]], on_update=list(si.on_update)
                    )
                    changed = True
                new.append(inst)
            if changed:
                bb.instructions = new
    return nc


_NC_CACHE = {}


def _get_nc():
    if "nc" not in _NC_CACHE:
        _NC_CACHE["nc"] = _legalize_waits(build_bass())
    return _NC_CACHE["nc"]


def prep_core_inputs(x_s, W_s, b_s):
    """Per-core shard [*, NPC nodes] -> device-layout arrays (padded)."""
    xs = np.zeros((B, H, NPAD), np.float16)
    xs[:, :, :NPC] = x_s
    Ws = np.zeros((NPAD, O, H), np.float32)
    Ws[:NPC] = W_s
    bs = np.zeros((NPAD, O), np.float32)
    bs[:NPC] = b_s

    # W slab [128, WCOLS]: [32k+h, 32s+o] = W[4s+k, o, h]
    wslab = np.ascontiguousarray(
        Ws.reshape(NG, 4, O, H).transpose(1, 3, 0, 2).reshape(128, WCOLS)
    ).astype(np.float16)

    # x slab [4, 32, NG*B]: [k, h, 16*s + b] = x[b, h, 4s+k]
    xd = np.ascontiguousarray(
        xs.reshape(B, H, NG, 4).transpose(3, 1, 2, 0).reshape(4, 32, NG * B)
    )

    # bias slab [128, NG]: [32c+o, 4*g4+k] = b[16g4+4c+k, o]
    bslab = np.ascontiguousarray(
        bs.reshape(NSUP, 4, 4, O).transpose(1, 3, 0, 2).reshape(128, NG)
    ).astype(np.float16)

    return {"x": xd, "W": wslab, "b": bslab}


def unprep_core_output(op):
    """Device out slab [128, OUTCOLS] fp16 -> [B, O, NPC] f32."""
    # [32c+o, 64*g4 + 16k + b] = out[b, o, 16g4+4c+k]
    arr = np.asarray(op).reshape(4, O, NSUP, 4, B).transpose(4, 1, 2, 0, 3)
    return arr.reshape(B, O, NPAD)[:, :, :NPC].astype(np.float32)


def make_in_maps(x, W, b):
    x = np.ascontiguousarray(x, dtype=np.float32)
    W = np.ascontiguousarray(W, dtype=np.float32)
    b = np.ascontiguousarray(b, dtype=np.float32)
    in_maps = []
    for core in range(NCORES):
        sl = slice(core * NPC, (core + 1) * NPC)
        in_maps.append(prep_core_inputs(x[:, :, sl], W[sl], b[sl]))
    return in_maps


def run_spmd(in_maps, **kwargs):
    from concourse.bass_utils import run_bass_kernel_spmd

    nc = _get_nc()
    return run_bass_kernel_spmd(
        nc, in_maps, core_ids=list(range(NCORES)), **kwargs
    )


def kernel(x, W, b):
    res = run_spmd(make_in_maps(x, W, b))
    out = np.concatenate(
        [unprep_core_output(res.results[c]["out"]) for c in range(NCORES)],
        axis=2,
    )
    return out
